# revision 1
# baseline (speedup 1.0000x reference)
"""Trainium2 Bass kernel for HematoxylinFFT: color-deconv H channel -> fft2
magnitude spectrum -> log1p -> per-image min-max norm -> InstanceNorm2d.

Data parallel over batch: 64 images sharded 8 per NeuronCore across 8 cores.
FFT2 realized as two fp32r matmul stages against a 512-pt shifted DFT matrix
(fftshift + 1/N forward norm baked into the matrix); all normalizations fused
into per-image scalar stats + one affine pass.
"""
import sys
sys.path.insert(0, "/opt/trn_rl_repo")
import numpy as np
from contextlib import ExitStack

import concourse.bass as bass
import concourse.bass_isa as bass_isa
import concourse.tile as tile
from concourse import bacc, mybir
from concourse.bass_utils import run_bass_kernel_spmd
from concourse import library_config

N = 512
NCORES = 8
BPC = 8  # images per core
DT = mybir.dt.float32
DTR = mybir.dt.float32r

# hematoxylin weights: first column of inv(rgb_from_hed), scaled by 1/log(1e-6)
_RGB_FROM_HED = np.array([[0.65, 0.70, 0.29],
                          [0.07, 0.99, 0.11],
                          [0.27, 0.57, 0.78]])
_W = np.linalg.inv(_RGB_FROM_HED).astype(np.float32)[:, 0]
_LA = float(np.log(1e-6))
_V = (_W / _LA).astype(np.float32)          # v0 < 0
_NT = float(N * N)


def _dft_consts():
    # shifted DFT: column i of GT corresponds to shifted freq i, k=(i+256)%N
    k = (np.arange(N) + 256) % N
    ang = -2.0 * np.pi * np.outer(np.arange(N), k) / N   # [n, i]
    gtr = (np.cos(ang) / N).astype(np.float32)
    gti = (np.sin(ang) / N).astype(np.float32)
    return gtr, gti, (-gti).astype(np.float32)


def _blk(ap):
    # [512, 512] dram view -> [128, 4, 512] (partition, row-block, col)
    return ap.rearrange("(blk p) w -> p blk w", p=128)


def _build_nc():
    nc = bacc.Bacc("TRN2", target_bir_lowering=False)
    x_d = nc.declare_dram_parameter("x", [BPC, 3, N, N], DT, isOutput=False)
    g_d = nc.declare_dram_parameter("gamma", [1], DT, isOutput=False)
    b_d = nc.declare_dram_parameter("beta", [1], DT, isOutput=False)
    gtr_d = nc.declare_dram_parameter("gtr", [N, N], DT, isOutput=False)
    gti_d = nc.declare_dram_parameter("gti", [N, N], DT, isOutput=False)
    gtin_d = nc.declare_dram_parameter("gtin", [N, N], DT, isOutput=False)
    y_d = nc.declare_dram_parameter("y", [BPC, 1, N, N], DT, isOutput=True)

    FL = 4 * N  # 2048 flat free size

    with tile.TileContext(nc) as tc:
        with ExitStack() as ctx:
            const_pool = ctx.enter_context(tc.tile_pool(name="consts", bufs=1))
            xt_pool = ctx.enter_context(tc.tile_pool(name="xt", bufs=6))
            t_pool = ctx.enter_context(tc.tile_pool(name="t", bufs=2))
            h_pool = ctx.enter_context(tc.tile_pool(name="h", bufs=2))
            yt_pool = ctx.enter_context(tc.tile_pool(name="yt", bufs=2))
            big_pool = ctx.enter_context(tc.tile_pool(name="big", bufs=1))
            lm_pool = ctx.enter_context(tc.tile_pool(name="lm", bufs=1))
            st_pool = ctx.enter_context(tc.tile_pool(name="st", bufs=24))
            ps1 = ctx.enter_context(tc.tile_pool(name="ps1", bufs=2, space="PSUM"))
            ps2 = ctx.enter_context(tc.tile_pool(name="ps2", bufs=2, space="PSUM"))

            nc.gpsimd.load_library(library_config.attn)
            # ---- constants: load fp32, round once to fp32r ----
            cr = {}
            for nm, d in (("gtr", gtr_d), ("gti", gti_d), ("gtin", gtin_d)):
                raw = xt_pool.tile([128, FL], DT, tag="xt")
                nc.sync.dma_start(raw[:].rearrange("p (a b) -> p a b", a=4), _blk(d[:, :]))
                r = const_pool.tile([128, FL], DTR, tag=f"c_{nm}")
                nc.vector.tensor_copy(r[:], raw[:])
                cr[nm] = r

            g_t = st_pool.tile([1, 1], DT, tag="gm")
            nc.sync.dma_start(g_t[:], g_d[:].unsqueeze(1))
            b_t = st_pool.tile([1, 1], DT, tag="bt")
            nc.sync.dma_start(b_t[:], b_d[:].unsqueeze(1))
            gb128 = const_pool.tile([128, 1], DT, tag="gb128")
            nc.gpsimd.partition_broadcast(gb128[:], g_t[:])
            bb128 = const_pool.tile([128, 1], DT, tag="bb128")
            nc.gpsimd.partition_broadcast(bb128[:], b_t[:])

            for b in range(BPC):
                # ---- load 3 channels, clip, log ----
                ls = []
                for c in range(3):
                    xt = xt_pool.tile([128, FL], DT, tag="xt")
                    nc.sync.dma_start(
                        xt[:].rearrange("p (a b) -> p a b", a=4), _blk(x_d[b, c])
                    )
                    nc.vector.tensor_scalar_max(xt[:], xt[:], 1e-6)
                    nc.scalar.activation(xt[:], xt[:], mybir.ActivationFunctionType.Ln)
                    ls.append(xt)
                # ---- channel combine + relu -> h (fp32r) ----
                t1 = t_pool.tile([128, FL], DT, tag="t")
                nc.vector.scalar_tensor_tensor(
                    t1[:], ls[1][:], float(_V[1] / _V[0]), ls[0][:],
                    mybir.AluOpType.mult, mybir.AluOpType.add,
                )
                t2 = t_pool.tile([128, FL], DT, tag="t")
                nc.vector.scalar_tensor_tensor(
                    t2[:], ls[2][:], float(_V[2] / _V[0]), t1[:],
                    mybir.AluOpType.mult, mybir.AluOpType.add,
                )
                h = h_pool.tile([128, FL], DTR, tag="h")
                nc.vector.tensor_scalar(
                    h[:], t2[:], float(_V[0]), 0.0,
                    mybir.AluOpType.mult, mybir.AluOpType.max,
                )

                # ---- stage 1: Yt[w, i] = sum_h h[h,w] G[i,h]  ----
                ytr = yt_pool.tile([128, FL], DTR, tag="ytr")
                yti = yt_pool.tile([128, FL], DTR, tag="yti")
                for m in range(4):
                    pr = ps1.tile([128, N], DT, tag="ytr")
                    pi = ps1.tile([128, N], DT, tag="yti")
                    for k in range(4):
                        lhs = h[:, k * N + m * 128: k * N + m * 128 + 128]
                        nc.tensor.matmul(pr[:], lhs, cr["gtr"][:, k * N:(k + 1) * N],
                                         start=(k == 0), stop=(k == 3))
                        nc.tensor.matmul(pi[:], lhs, cr["gti"][:, k * N:(k + 1) * N],
                                         start=(k == 0), stop=(k == 3))
                    nc.vector.tensor_copy(ytr[:, m * N:(m + 1) * N], pr[:])
                    nc.vector.tensor_copy(yti[:, m * N:(m + 1) * N], pi[:])

                # ---- stage 2 + squares ----
                sqr = big_pool.tile([128, FL], DT, tag="sqr")
                sqi = big_pool.tile([128, FL], DT, tag="sqi")
                for mi in range(4):
                    zr = ps2.tile([128, N], DT, tag="zr")
                    zi = ps2.tile([128, N], DT, tag="zi")
                    for k in range(4):
                        lr = ytr[:, k * N + mi * 128: k * N + mi * 128 + 128]
                        li = yti[:, k * N + mi * 128: k * N + mi * 128 + 128]
                        first, last = (k == 0), (k == 3)
                        nc.tensor.matmul(zr[:], lr, cr["gtr"][:, k * N:(k + 1) * N],
                                         start=first, stop=False)
                        nc.tensor.matmul(zi[:], lr, cr["gti"][:, k * N:(k + 1) * N],
                                         start=first, stop=False)
                        nc.tensor.matmul(zr[:], li, cr["gtin"][:, k * N:(k + 1) * N],
                                         start=False, stop=last)
                        nc.tensor.matmul(zi[:], li, cr["gtr"][:, k * N:(k + 1) * N],
                                         start=False, stop=last)
                    nc.scalar.square(sqr[:, mi * N:(mi + 1) * N], zr[:])
                    nc.scalar.square(sqi[:, mi * N:(mi + 1) * N], zi[:])

                # ---- m2 (+ running max), min, sqrt, log1p (+S1), S2 ----
                m2 = big_pool.tile([128, FL], DT, tag="m2")
                mx = st_pool.tile([128, 1], DT, tag="mx")
                nc.vector.tensor_add(m2[:], sqr[:], sqi[:])
                nc.vector.tensor_reduce(mx[:], m2[:], mybir.AxisListType.X,
                                        mybir.AluOpType.max)
                mn = st_pool.tile([128, 1], DT, tag="mn")
                nc.vector.tensor_reduce(mn[:], m2[:], mybir.AxisListType.X,
                                        mybir.AluOpType.min)
                mg = big_pool.tile([128, FL], DT, tag="mg")
                nc.scalar.sqrt(mg[:], m2[:])
                lm = lm_pool.tile([128, FL], DT, tag="lm")
                s1p = st_pool.tile([128, 1], DT, tag="s1p")
                nc.scalar.activation(lm[:], mg[:], mybir.ActivationFunctionType.Ln,
                                     bias=1.0, accum_out=s1p[:])
                junk = big_pool.tile([128, FL], DT, tag="sqr")
                s2p = st_pool.tile([128, 1], DT, tag="s2p")
                nc.vector.tensor_mul(junk[:], lm[:], lm[:])
                nc.vector.tensor_reduce(s2p[:], junk[:], mybir.AxisListType.X,
                                        mybir.AluOpType.add)

                # ---- cross-partition stats: all-reduce -> same value on all
                # 128 partitions, then do scalar math on [128,1] lanes ----
                AF = mybir.ActivationFunctionType
                RO = bass_isa.ReduceOp
                mxr = st_pool.tile([128, 1], DT, tag="mxr")
                nc.gpsimd.partition_all_reduce(mxr[:], mx[:], 128, RO.max)
                nmn = st_pool.tile([128, 1], DT, tag="nmn")
                nc.vector.tensor_scalar_mul(nmn[:], mn[:], -1.0)
                nmnr = st_pool.tile([128, 1], DT, tag="nmnr")
                nc.gpsimd.partition_all_reduce(nmnr[:], nmn[:], 128, RO.max)
                s1r = st_pool.tile([128, 1], DT, tag="s1r")
                nc.gpsimd.partition_all_reduce(s1r[:], s1p[:], 128, RO.add)
                s2r = st_pool.tile([128, 1], DT, tag="s2r")
                nc.gpsimd.partition_all_reduce(s2r[:], s2p[:], 128, RO.add)

                # lmx/lmn = log1p(sqrt(.)), rng^2
                lmx = st_pool.tile([128, 1], DT, tag="lmx")
                nc.scalar.sqrt(lmx[:], mxr[:])
                nc.scalar.activation(lmx[:], lmx[:], AF.Ln, bias=1.0)
                lmn = st_pool.tile([128, 1], DT, tag="lmn")
                nc.scalar.activation(lmn[:], nmnr[:], AF.Sqrt, scale=-1.0)
                nc.scalar.activation(lmn[:], lmn[:], AF.Ln, bias=1.0)
                rg = st_pool.tile([128, 1], DT, tag="rg")
                nc.vector.tensor_sub(rg[:], lmx[:], lmn[:])
                r2 = st_pool.tile([128, 1], DT, tag="r2")
                nc.vector.tensor_mul(r2[:], rg[:], rg[:])
                # mu, E2, var, d = var + 1e-5*rng^2
                mu = st_pool.tile([128, 1], DT, tag="mu")
                nc.vector.tensor_scalar_mul(mu[:], s1r[:], 1.0 / _NT)
                e2 = st_pool.tile([128, 1], DT, tag="e2")
                nc.vector.tensor_scalar_mul(e2[:], s2r[:], 1.0 / _NT)
                msq = st_pool.tile([128, 1], DT, tag="msq")
                nc.vector.tensor_mul(msq[:], mu[:], mu[:])
                var = st_pool.tile([128, 1], DT, tag="var")
                nc.vector.tensor_sub(var[:], e2[:], msq[:])
                d = st_pool.tile([128, 1], DT, tag="d")
                nc.vector.scalar_tensor_tensor(
                    d[:], r2[:], 1e-5, var[:],
                    mybir.AluOpType.mult, mybir.AluOpType.add,
                )
                sd = st_pool.tile([128, 1], DT, tag="sd")
                nc.scalar.sqrt(sd[:], d[:])
                inv = st_pool.tile([128, 1], DT, tag="inv")
                nc.vector.reciprocal(inv[:], sd[:])
                sv = st_pool.tile([128, 1], DT, tag="sv")
                nc.vector.tensor_mul(sv[:], inv[:], gb128[:])
                nmu = st_pool.tile([128, 1], DT, tag="nmu")
                nc.vector.tensor_scalar_mul(nmu[:], mu[:], -1.0)
                bv = st_pool.tile([128, 1], DT, tag="bv")
                nc.vector.scalar_tensor_tensor(
                    bv[:], nmu[:], sv[:], bb128[:],
                    mybir.AluOpType.mult, mybir.AluOpType.add,
                )

                # ---- affine + store ----
                o = lm_pool.tile([128, FL], DT, tag="o")
                nc.scalar.activation(o[:], lm[:], AF.Identity,
                                     bias=bv[:], scale=sv[:])
                nc.sync.dma_start(
                    _blk(y_d[b, 0]), o[:].rearrange("p (a b) -> p a b", a=4)
                )

    nc.finalize()
    return nc


_NC_CACHE = None


def kernel(x, gamma, beta):
    global _NC_CACHE
    if _NC_CACHE is None:
        _NC_CACHE = _build_nc()
    nc = _NC_CACHE
    x = np.ascontiguousarray(np.asarray(x, dtype=np.float32))
    gamma = np.asarray(gamma, dtype=np.float32)
    beta = np.asarray(beta, dtype=np.float32)
    gtr, gti, gtin = _dft_consts()
    in_maps = [
        {"x": x[c * BPC:(c + 1) * BPC], "gamma": gamma, "beta": beta,
         "gtr": gtr, "gti": gti, "gtin": gtin}
        for c in range(NCORES)
    ]
    res = run_bass_kernel_spmd(nc, in_maps, list(range(NCORES)))
    out = np.concatenate([res.results[i]["y"] for i in range(NCORES)], axis=0)
    return out.astype(np.float32)

